# revision 1
# baseline (speedup 1.0000x reference)
"""Trainium2 Bass kernel for Brain3DQTUNNetwork (gnn_message_passing).

The "sparse" graph is a fixed Manhattan-radius-2 stencil on a 64^3 grid
(24 offsets).  Weights are stored dense per offset slot, keyed by the
DESTINATION (col) index: W[k][j] = w(edge j-d_k -> j), 0 for invalid
edges.  The per-step segment_sum SpMV becomes 24 shifted elementwise
multiply-accumulates, and the STDP update becomes
    w = clip(w*(1-WDECAY) + prev * q_shift,  0, 1),   q(o) = 0.015*o - 0.005.
Invalid slots self-heal to 0 every step (q reads 0 / q(0) < 0 there and
the clip floors at 0).

Layout: partition axis = x (64).  Free axis = (y_local + halo, z + pad):
12*68 = 816.  y/z shifts are free-dim AP offsets; x shifts are realized
as 4 SBUF->SBUF DMA partition-shifted copies (engine APs must start at a
32-aligned partition, DMA has no such constraint).

Sharding: 8 y-slabs of 8 y-planes each.  All state (weights, v, prev)
stays SBUF-resident for all 50 steps; per-step cross-core traffic is a
single 8-rank AllGather of the 2-row boundary strips, with neighbor
extraction via partition-id-driven dynamic DMA.
"""

import os
import sys

sys.path.insert(0, "/opt/trn_rl_repo")

import numpy as np

import concourse.bass as bass
import concourse.bacc as bacc
import concourse.mybir as mybir
import concourse.tile as tile
from concourse import bass_utils

# ---- problem constants (hardcoded; kernel.py must be self-contained) ----
GRID = (64, 64, 64)
NX, NY, NZ = GRID
N = NX * NY * NZ
RADIUS = 2
NCORES = 8
YS = NY // NCORES  # y-planes per core = 8

TAU = 20.0
REST_V = -65.0
EXC_THR = -50.0
INH_THR = -70.0
RESET_V = -65.0
ETA_LTP, ETA_LTD, WDECAY = 0.01, 0.005, 1e-05

# fp32-exact scalars matching the jax reference
DECAY = float(np.exp(np.float32(-1.0 / np.float32(TAU))).astype(np.float32))
ONE_MINUS_DECAY = float(np.float32(1.0) - np.float32(DECAY))
MIDPOINT = (EXC_THR + INH_THR) / 2.0  # -60.0

# offsets sorted by descending linear shift (matches reference edge order:
# for a destination j, contributions are summed over ascending source i)
OFFSETS = sorted(
    [
        (dx, dy, dz)
        for dx in range(-RADIUS, RADIUS + 1)
        for dy in range(-RADIUS, RADIUS + 1)
        for dz in range(-RADIUS, RADIUS + 1)
        if 0 < abs(dx) + abs(dy) + abs(dz) <= RADIUS
    ],
    key=lambda d: -(d[0] * NY * NZ + d[1] * NZ + d[2]),
)
NOFF = len(OFFSETS)  # 24
XSHIFTS = (1, -1, 2, -2)

# field geometry: partition p = x (64), free = ys*68 + zs,
# ys = y_loc + 2 in [0,12), zs = z + 2 in [0,68)
FYS = 12           # field y rows (8 own + 2 halo each side)
FZS = 68           # field z cols (64 + 2 pad each side)
FFREE = FYS * FZS  # 816
CHUNK = YS * NZ    # 512 elems per partition for compact tiles
F32 = mybir.dt.float32

_CACHE = {}


def _build_graph(nsteps):
    nc = bacc.Bacc(
        "TRN2",
        target_bir_lowering=False,
        debug=False,
        enable_asserts=True,
        num_devices=NCORES,
    )
    w0_d = nc.dram_tensor("w0", [NX, NOFF * CHUNK], F32, kind="ExternalInput").ap()
    xin_d = nc.dram_tensor("xin", [nsteps, NX, CHUNK], F32, kind="ExternalInput").ap()
    msk_d = nc.dram_tensor("msk", [NX, 2], F32, kind="ExternalInput").ap()
    spk_d = nc.dram_tensor("spk", [nsteps, NX, CHUNK], F32, kind="ExternalOutput").ap()

    AT = mybir.ActivationFunctionType
    ALU = mybir.AluOpType

    with tile.TileContext(nc) as tc, tc.tile_pool(
        name="state", bufs=1
    ) as st, tc.tile_pool(name="dram", bufs=1, space="DRAM") as dr:
        # persistent state tiles (distinct names -> distinct slots)
        W = st.tile([NX, NOFF * CHUNK], F32, name="W")
        P = st.tile([NX, NOFF * CHUNK], F32, name="P")
        FA = st.tile([NX, FFREE], F32, name="FA")
        FB = st.tile([NX, FFREE], F32, name="FB")
        QF = st.tile([NX, FFREE], F32, name="QF")
        SO = {dx: st.tile([NX, FFREE], F32, name=f"SOx{dx+2}") for dx in XSHIFTS}
        SQ = {dx: st.tile([NX, FFREE], F32, name=f"SQx{dx+2}") for dx in XSHIFTS}
        V = st.tile([NX, CHUNK], F32, name="V")
        SYN = st.tile([NX, CHUNK], F32, name="SYN")
        S = st.tile([NX, CHUNK], F32, name="S")
        II = st.tile([NX, CHUNK], F32, name="II")
        G = st.tile([NX, CHUNK], F32, name="G")
        NN = st.tile([NX, CHUNK], F32, name="NN")
        T1 = st.tile([NX, CHUNK], F32, name="T1")
        XIP = st.tile([NX, CHUNK], F32, name="XIP")
        RST = st.tile([NX, CHUNK], F32, name="RST")
        MSK = st.tile([NX, 2], F32, name="MSK")
        SND = st.tile([NX, 4 * NZ], F32, name="SND")
        B30 = st.tile([NX, 1], F32, name="B30")  # sigmoid bias const
        SI = st.tile([NX, CHUNK], mybir.dt.uint8, name="SI")  # int spike mask

        def f3(t):  # [NX, FFREE] -> [NX, FYS, FZS]
            return t.rearrange("p (y z) -> p y z", z=FZS)

        def c3(t):  # [NX, CHUNK] -> [NX, YS, NZ]
            return t.rearrange("p (y z) -> p y z", z=NZ)

        def w3(t, k):  # [NX, NOFF*CHUNK] slot k -> [NX, YS, NZ]
            return t.rearrange("p (k y z) -> p k y z", k=NOFF, z=NZ)[:, k]

        def fint(t):  # own interior of a field tile -> [NX, YS, NZ]
            return f3(t)[:, 2 : 2 + YS, 2 : 2 + NZ]

        def fshift(base, xs, d):
            # source view reading position j - d, with x-shift via copies
            dx, dy, dz = d
            t = base if dx == 0 else xs[dx]
            return f3(t)[:, 2 - dy : 2 - dy + YS, 2 - dz : 2 - dz + NZ]

        def xshift_dma(dst, src, dx):
            # dst[p] = src[p - dx] for the valid range (edge rows stay 0)
            a, b = max(0, dx), NX + min(0, dx)
            nc.sync.dma_start(dst[a:b, :], src[a - dx : b - dx, :])

        # ---- init ----
        nc.vector.memset(FA[:], 0.0)
        nc.vector.memset(FB[:], 0.0)
        nc.vector.memset(QF[:], 0.0)
        for dx in XSHIFTS:
            nc.vector.memset(SO[dx][:], 0.0)
            nc.vector.memset(SQ[dx][:], 0.0)
        nc.vector.memset(V[:], REST_V)
        nc.vector.memset(B30[:], -0.5 * MIDPOINT)
        nc.vector.memset(RST[:], RESET_V)
        nc.sync.dma_start(W[:], w0_d[:])
        nc.sync.dma_start(MSK[:], msk_d[:])

        pid = nc.sync.partition_id()
        offL = nc.sync.snap((pid + NCORES - 1) % NCORES, min_val=0, max_val=NCORES - 1)
        offR = nc.sync.snap((pid + 1) % NCORES, min_val=0, max_val=NCORES - 1)

        fields = [FA, FB]
        for t in range(nsteps):
            FPREV = fields[t % 2]       # holds out_{t-1} (with halos)
            FOUT = fields[(t + 1) % 2]  # will hold out_t

            # external input, prescaled by (1-decay)
            nc.sync.dma_start(XIP[:], xin_d[t])
            nc.scalar.mul(XIP[:], XIP[:], ONE_MINUS_DECAY)

            # ---- syn: 24 shifted products then pairwise tree sum ----
            for k, d in enumerate(OFFSETS):
                nc.vector.tensor_tensor(
                    w3(P, k), w3(W, k), fshift(FPREV, SO, d), ALU.mult
                )
            # tree-reduce the 24 slots (in place over P)
            Pf = P.rearrange("p (k f) -> p k f", k=NOFF)
            nslots = NOFF
            while nslots > 1:
                half = nslots // 2
                nc.vector.tensor_tensor(
                    Pf[:, 0:half],
                    Pf[:, 0:half],
                    Pf[:, half : 2 * half],
                    ALU.add,
                )
                if nslots % 2:
                    nc.vector.tensor_tensor(
                        Pf[:, 0], Pf[:, 0], Pf[:, nslots - 1], ALU.add
                    )
                nslots = half
            # SYN = (P0 * (1-decay)) + XIP ; then v = v*decay + SYN
            nc.vector.scalar_tensor_tensor(
                SYN[:], Pf[:, 0], ONE_MINUS_DECAY, XIP[:], ALU.mult, ALU.add
            )
            nc.vector.scalar_tensor_tensor(
                V[:], V[:], DECAY, SYN[:], ALU.mult, ALU.add
            )

            # ---- neuron update ----
            nc.gpsimd.tensor_single_scalar(S[:], V[:], EXC_THR, ALU.is_ge)
            nc.gpsimd.tensor_single_scalar(SI[:], V[:], EXC_THR, ALU.is_ge)
            nc.gpsimd.tensor_single_scalar(II[:], V[:], INH_THR, ALU.is_le)
            nc.scalar.activation(G[:], V[:], AT.Sigmoid, bias=B30[:, 0:1], scale=0.5)
            nc.gpsimd.tensor_tensor(NN[:], S[:], II[:], ALU.add)
            nc.vector.scalar_tensor_tensor(
                T1[:], NN[:], 1.0, G[:], ALU.subtract, ALU.mult
            )
            # out = s - (n-1)*g, written straight into the FOUT interior
            nc.vector.tensor_tensor(fint(FOUT), c3(S), c3(T1), ALU.subtract)
            # v reset where spiked
            nc.vector.copy_predicated(V[:], SI[:], RST[:])
            # spike train output for this step
            nc.sync.dma_start(spk_d[t], S[:])

            # ---- halo exchange of out_t ----
            snd3 = SND.rearrange("p (y z) -> p y z", z=NZ)
            # top 2 own rows (y_loc 6,7) masked by col-0, bottom 2 (y_loc 0,1) by col-1
            nc.scalar.activation(
                snd3[:, 0:2], f3(FOUT)[:, 8:10, 2 : 2 + NZ],
                AT.Copy, bias=0.0, scale=MSK[:, 0:1],
            )
            nc.scalar.activation(
                snd3[:, 2:4], f3(FOUT)[:, 2:4, 2 : 2 + NZ],
                AT.Copy, bias=0.0, scale=MSK[:, 1:2],
            )
            agin = dr.tile([NX, 4 * NZ], F32, name=f"agin{t}")
            agout = dr.tile(
                [NCORES * NX, 4 * NZ], F32, addr_space="Shared", name=f"agout{t}"
            )
            nc.sync.dma_start(agin[:], SND[:])
            nc.gpsimd.collective_compute(
                "AllGather",
                ALU.bypass,
                replica_groups=[list(range(NCORES))],
                ins=[agin.opt()],
                outs=[agout.opt()],
            )
            agf = agout.rearrange("p (y z) -> p y z", z=NZ)
            # left neighbor's top strip -> my bottom halo rows (ys 0,1)
            nc.sync.dma_start(
                f3(FOUT)[:, 0:2, 2 : 2 + NZ],
                agf[bass.ds(offL * NX, NX), 0:2],
            )
            # right neighbor's bottom strip -> my top halo rows (ys 10,11)
            nc.sync.dma_start(
                f3(FOUT)[:, 10:12, 2 : 2 + NZ],
                agf[bass.ds(offR * NX, NX), 2:4],
            )

            # x-shifted copies of out_t (serve step t+1 syn and step t STDP)
            for dx in XSHIFTS:
                xshift_dma(SO[dx], FOUT, dx)

            # ---- STDP (skipped at t=0, matching the reference) ----
            if t > 0:
                # q = 0.015*out_t - 0.005 over the full halo'd field
                nc.scalar.activation(
                    QF[:], FOUT[:], AT.Copy, bias=-ETA_LTD, scale=ETA_LTP + ETA_LTD
                )
                for dx in XSHIFTS:
                    xshift_dma(SQ[dx], QF, dx)
                for k, d in enumerate(OFFSETS):
                    nc.vector.tensor_tensor(
                        w3(P, k), fint(FPREV), fshift(QF, SQ, d), ALU.mult
                    )
                nc.vector.scalar_tensor_tensor(
                    W[:], W[:], 1.0 - WDECAY, P[:], ALU.mult, ALU.add
                )
                nc.vector.tensor_scalar(W[:], W[:], 1.0, 0.0, ALU.min, ALU.max)

    nc.compile()
    return nc


def _shard_inputs(external_input, edge_values, edge_rows, edge_cols, nsteps):
    """Build per-core input maps (host-side sharding)."""
    ext = np.ascontiguousarray(np.asarray(external_input, dtype=np.float32))[:nsteps]
    vals = np.asarray(edge_values, dtype=np.float32)
    rows = np.asarray(edge_rows, dtype=np.int64)
    cols = np.asarray(edge_cols, dtype=np.int64)

    # dense weights keyed by destination: Wd[k, j] = w(edge j-d_k -> j)
    dlin = cols - rows
    offs_lin = np.array([d[0] * NY * NZ + d[1] * NZ + d[2] for d in OFFSETS])
    assert set(int(v) for v in np.unique(dlin)).issubset(
        set(int(v) for v in offs_lin)
    )
    k_of = np.zeros(int(offs_lin.max()) - int(offs_lin.min()) + 1, dtype=np.int64)
    for i, v in enumerate(offs_lin):
        k_of[int(v) - int(offs_lin.min())] = i
    ke = k_of[dlin - int(offs_lin.min())]
    Wd = np.zeros((NOFF, N), dtype=np.float32)
    Wd[ke, cols] = vals

    Wd = Wd.reshape(NOFF, NX, NY, NZ)
    ext = ext.reshape(nsteps, NX, NY, NZ)

    in_maps = []
    for c in range(NCORES):
        ylo = c * YS
        wc = np.ascontiguousarray(
            Wd[:, :, ylo : ylo + YS, :].transpose(1, 0, 2, 3)
        ).reshape(NX, NOFF * CHUNK)
        xc = np.ascontiguousarray(ext[:, :, ylo : ylo + YS, :]).reshape(
            nsteps, NX, CHUNK
        )
        msk = np.zeros((NX, 2), dtype=np.float32)
        msk[:, 0] = 0.0 if c == NCORES - 1 else 1.0  # top strip valid?
        msk[:, 1] = 0.0 if c == 0 else 1.0           # bottom strip valid?
        in_maps.append({"w0": wc, "xin": xc, "msk": msk})
    return in_maps


def kernel(external_input, edge_values, edge_rows, edge_cols, num_steps):
    nsteps = int(num_steps)
    if nsteps not in _CACHE:
        _CACHE[nsteps] = _build_graph(nsteps)
    nc = _CACHE[nsteps]

    in_maps = _shard_inputs(external_input, edge_values, edge_rows, edge_cols, nsteps)
    res = bass_utils.run_bass_kernel_spmd(
        nc,
        in_maps,
        core_ids=list(range(NCORES)),
        trace=bool(int(os.environ.get("BRAIN_TRACE", "0"))),
    )

    out = np.empty((nsteps, NX, NY, NZ), dtype=np.float32)
    for c in range(NCORES):
        ylo = c * YS
        out[:, :, ylo : ylo + YS, :] = res.results[c]["spk"].reshape(
            nsteps, NX, YS, NZ
        )
    kernel.last_results = res
    return out.reshape(nsteps, N)



# revision 2
# speedup vs baseline: 1.5919x; 1.5919x over previous
"""Trainium2 Bass kernel for Brain3DQTUNNetwork (gnn_message_passing).

The "sparse" graph is a fixed Manhattan-radius-2 stencil on a 64^3 grid
(24 offsets).  Weights are stored dense per offset slot, keyed by the
DESTINATION (col) index: W[k][j] = w(edge j-d_k -> j), 0 for invalid
edges.  The per-step segment_sum SpMV becomes 24 shifted elementwise
multiply-accumulates, and the STDP update becomes
    w = clip(w*(1-WDECAY) + prev * q_shift,  0, 1),   q(o) = 0.015*o - 0.005.
Invalid slots self-heal to 0 every step (q reads 0 / q(0) < 0 there and
the clip floors at 0).

Layout: partition axis = x (64).  Free axis = (y_local + halo, z + pad):
12*68 = 816.  y/z shifts are free-dim AP offsets; x shifts are realized
as 4 SBUF->SBUF DMA partition-shifted copies (engine APs must start at a
32-aligned partition, DMA has no such constraint).

Sharding: 8 y-slabs of 8 y-planes each.  All state (weights, v, prev)
stays SBUF-resident for all 50 steps; per-step cross-core traffic is a
single 8-rank AllGather of the 2-row boundary strips, with neighbor
extraction via partition-id-driven dynamic DMA.

Engine assignment: all elementwise math on Vector (DVE); sigmoid, mask
copies on Scalar; GpSimd only triggers the collective.  The STDP
product uses the fused affine_mul_reduce DVE op, which folds q() into
the multiply so no shifted copies of q(out) are ever materialized.
Offset slots are ordered dy==0 first so the first half of the STDP/W
update (which needs no y-halo) overlaps the AllGather latency.
"""

import os
import sys

sys.path.insert(0, "/opt/trn_rl_repo")

import numpy as np

import concourse.bass as bass
import concourse.bacc as bacc
import concourse.mybir as mybir
import concourse.tile as tile
from concourse import bass_utils

# ---- problem constants (hardcoded; kernel.py must be self-contained) ----
GRID = (64, 64, 64)
NX, NY, NZ = GRID
N = NX * NY * NZ
RADIUS = 2
NCORES = 8
YS = NY // NCORES  # y-planes per core = 8

TAU = 20.0
REST_V = -65.0
EXC_THR = -50.0
INH_THR = -70.0
RESET_V = -65.0
ETA_LTP, ETA_LTD, WDECAY = 0.01, 0.005, 1e-05

# fp32-exact scalars matching the jax reference
DECAY = float(np.exp(np.float32(-1.0 / np.float32(TAU))).astype(np.float32))
ONE_MINUS_DECAY = float(np.float32(1.0) - np.float32(DECAY))
MIDPOINT = (EXC_THR + INH_THR) / 2.0  # -60.0

# offset slots: dy == 0 first (no y-halo needed -> can update W while the
# halo AllGather is in flight), then dy != 0.  Within each group, sorted
# by descending linear shift.
_ALL_OFFS = [
    (dx, dy, dz)
    for dx in range(-RADIUS, RADIUS + 1)
    for dy in range(-RADIUS, RADIUS + 1)
    for dz in range(-RADIUS, RADIUS + 1)
    if 0 < abs(dx) + abs(dy) + abs(dz) <= RADIUS
]
_key = lambda d: -(d[0] * NY * NZ + d[1] * NZ + d[2])
OFFSETS = sorted([d for d in _ALL_OFFS if d[1] == 0], key=_key) + sorted(
    [d for d in _ALL_OFFS if d[1] != 0], key=_key
)
NOFF = len(OFFSETS)  # 24
NOFF_A = sum(1 for d in OFFSETS if d[1] == 0)  # 12 (dy == 0 slots)
XSHIFTS = (1, -1, 2, -2)

# field geometry: partition p = x (64), free = ys*68 + zs,
# ys = y_loc + 2 in [0,12), zs = z + 2 in [0,68)
FYS = 12           # field y rows (8 own + 2 halo each side)
FZS = 68           # field z cols (64 + 2 pad each side)
FFREE = FYS * FZS  # 816
CHUNK = YS * NZ    # 512 elems per partition for compact tiles
F32 = mybir.dt.float32

_CACHE = {}


def _build_graph(nsteps):
    nc = bacc.Bacc(
        "TRN2",
        target_bir_lowering=False,
        debug=False,
        enable_asserts=True,
        num_devices=NCORES,
    )
    w0_d = nc.dram_tensor("w0", [NX, NOFF * CHUNK], F32, kind="ExternalInput").ap()
    xin_d = nc.dram_tensor("xin", [nsteps, NX, CHUNK], F32, kind="ExternalInput").ap()
    msk_d = nc.dram_tensor("msk", [NX, 2], F32, kind="ExternalInput").ap()
    spk_d = nc.dram_tensor("spk", [nsteps, NX, CHUNK], F32, kind="ExternalOutput").ap()

    AT = mybir.ActivationFunctionType
    ALU = mybir.AluOpType

    with tile.TileContext(nc) as tc, tc.tile_pool(
        name="state", bufs=1
    ) as st, tc.tile_pool(name="dram", bufs=1, space="DRAM") as dr:
        # persistent state tiles (distinct names -> distinct slots)
        W = st.tile([NX, NOFF * CHUNK], F32, name="W")
        P = st.tile([NX, NOFF * CHUNK], F32, name="P")
        FA = st.tile([NX, FFREE], F32, name="FA")
        FB = st.tile([NX, FFREE], F32, name="FB")
        SO = {dx: st.tile([NX, FFREE], F32, name=f"SOx{dx+2}") for dx in XSHIFTS}
        V = st.tile([NX, CHUNK], F32, name="V")
        SYN = st.tile([NX, CHUNK], F32, name="SYN")
        S = st.tile([NX, CHUNK], F32, name="S")
        II = st.tile([NX, CHUNK], F32, name="II")
        G = st.tile([NX, CHUNK], F32, name="G")
        NN = st.tile([NX, CHUNK], F32, name="NN")
        T1 = st.tile([NX, CHUNK], F32, name="T1")
        T2 = st.tile([NX, CHUNK], F32, name="T2")
        XIP = st.tile([NX, CHUNK], F32, name="XIP")
        MSK = st.tile([NX, 2], F32, name="MSK")
        SND = st.tile([NX, 4 * NZ], F32, name="SND")
        B30 = st.tile([NX, 1], F32, name="B30")  # sigmoid bias const
        ACC = st.tile([NX, NOFF], F32, name="ACC")  # affine_mul_reduce scratch

        def f3(t):  # [NX, FFREE] -> [NX, FYS, FZS]
            return t.rearrange("p (y z) -> p y z", z=FZS)

        def c3(t):  # [NX, CHUNK] -> [NX, YS, NZ]
            return t.rearrange("p (y z) -> p y z", z=NZ)

        def w3(t, k):  # [NX, NOFF*CHUNK] slot k -> [NX, YS, NZ]
            return t.rearrange("p (k y z) -> p k y z", k=NOFF, z=NZ)[:, k]

        def fint(t):  # own interior of a field tile -> [NX, YS, NZ]
            return f3(t)[:, 2 : 2 + YS, 2 : 2 + NZ]

        def fshift(base, xs, d):
            # source view reading position j - d, with x-shift via copies
            dx, dy, dz = d
            t = base if dx == 0 else xs[dx]
            return f3(t)[:, 2 - dy : 2 - dy + YS, 2 - dz : 2 - dz + NZ]

        def xshift_rows(dst, src, dx, r0, r1):
            # dst[p, r0:r1, :] = src[p - dx, r0:r1, :] (edge partitions stay 0)
            a, b = max(0, dx), NX + min(0, dx)
            nc.sync.dma_start(
                f3(dst)[a:b, r0:r1, :], f3(src)[a - dx : b - dx, r0:r1, :]
            )

        # ---- init ----
        nc.vector.memset(FA[:], 0.0)
        nc.vector.memset(FB[:], 0.0)
        for dx in XSHIFTS:
            nc.vector.memset(SO[dx][:], 0.0)
        nc.vector.memset(V[:], REST_V)
        nc.vector.memset(B30[:], -0.5 * MIDPOINT)
        nc.sync.dma_start(W[:], w0_d[:])
        nc.sync.dma_start(MSK[:], msk_d[:])

        pid = nc.sync.partition_id()
        offL = nc.sync.snap((pid + NCORES - 1) % NCORES, min_val=0, max_val=NCORES - 1)
        offR = nc.sync.snap((pid + 1) % NCORES, min_val=0, max_val=NCORES - 1)

        fields = [FA, FB]
        for t in range(nsteps):
            FPREV = fields[t % 2]       # holds out_{t-1} (with halos)
            FOUT = fields[(t + 1) % 2]  # will hold out_t

            # external input, prescaled by (1-decay) (scalar engine, off-path)
            nc.sync.dma_start(XIP[:], xin_d[t])
            nc.scalar.mul(XIP[:], XIP[:], ONE_MINUS_DECAY)

            # ---- syn: 24 shifted products then pairwise tree sum ----
            for k, d in enumerate(OFFSETS):
                nc.vector.tensor_tensor(
                    w3(P, k), w3(W, k), fshift(FPREV, SO, d), ALU.mult
                )
            # tree-reduce the 24 slots (in place over P)
            Pf = P.rearrange("p (k f) -> p k f", k=NOFF)
            nslots = NOFF
            while nslots > 1:
                half = nslots // 2
                nc.vector.tensor_tensor(
                    Pf[:, 0:half],
                    Pf[:, 0:half],
                    Pf[:, half : 2 * half],
                    ALU.add,
                )
                if nslots % 2:
                    nc.vector.tensor_tensor(
                        Pf[:, 0], Pf[:, 0], Pf[:, nslots - 1], ALU.add
                    )
                nslots = half
            # SYN = (P0 * (1-decay)) + XIP ; then v = v*decay + SYN
            nc.vector.scalar_tensor_tensor(
                SYN[:], Pf[:, 0], ONE_MINUS_DECAY, XIP[:], ALU.mult, ALU.add
            )
            nc.vector.scalar_tensor_tensor(
                V[:], V[:], DECAY, SYN[:], ALU.mult, ALU.add
            )

            # ---- neuron update (vector TS + scalar sigmoid) ----
            nc.scalar.activation(G[:], V[:], AT.Sigmoid, bias=B30[:, 0:1], scale=0.5)
            nc.vector.tensor_single_scalar(S[:], V[:], EXC_THR, ALU.is_ge)
            nc.vector.tensor_single_scalar(II[:], V[:], INH_THR, ALU.is_le)
            nc.vector.tensor_tensor(NN[:], S[:], II[:], ALU.add)
            nc.vector.scalar_tensor_tensor(
                T1[:], NN[:], 1.0, G[:], ALU.subtract, ALU.mult
            )
            # out = s - (n-1)*g, written straight into the FOUT interior
            nc.vector.tensor_tensor(fint(FOUT), c3(S), c3(T1), ALU.subtract)

            # ---- halo exchange of out_t: launch ASAP ----
            snd3 = SND.rearrange("p (y z) -> p y z", z=NZ)
            # top 2 own rows (y_loc 6,7) masked by col-0, bottom 2 (y_loc 0,1) by col-1
            nc.scalar.activation(
                snd3[:, 0:2], f3(FOUT)[:, 8:10, 2 : 2 + NZ],
                AT.Copy, bias=0.0, scale=MSK[:, 0:1],
            )
            nc.scalar.activation(
                snd3[:, 2:4], f3(FOUT)[:, 2:4, 2 : 2 + NZ],
                AT.Copy, bias=0.0, scale=MSK[:, 1:2],
            )
            agin = dr.tile([NX, 4 * NZ], F32, name=f"agin{t}")
            agout = dr.tile(
                [NCORES * NX, 4 * NZ], F32, addr_space="Shared", name=f"agout{t}"
            )
            nc.sync.dma_start(agin[:], SND[:])
            nc.gpsimd.collective_compute(
                "AllGather",
                ALU.bypass,
                replica_groups=[list(range(NCORES))],
                ins=[agin.opt()],
                outs=[agout.opt()],
            )

            # ---- off the critical path while the collective flies ----
            # v reset where spiked: v -= (v - RESET_V) * s
            nc.vector.scalar_tensor_tensor(
                T2[:], V[:], -RESET_V, S[:], ALU.add, ALU.mult
            )
            nc.vector.tensor_tensor(V[:], V[:], T2[:], ALU.subtract)
            # spike train output for this step
            nc.sync.dma_start(spk_d[t], S[:])
            # x-shifted copies of out_t, interior rows only (halo rows patched
            # after the exchange lands)
            for dx in XSHIFTS:
                xshift_rows(SO[dx], FOUT, dx, 2, 10)

            # ---- STDP part A: dy == 0 slots need no y-halo (t=0 skipped) ----
            # dw[k][j] = prev[j] * (0.015*out[j-dk] - 0.005) via fused DVE op
            if t > 0:
                for k, d in enumerate(OFFSETS[:NOFF_A]):
                    nc.vector.affine_mul_reduce(
                        w3(P, k),
                        ACC[:, k : k + 1],
                        fshift(FOUT, SO, d),
                        fint(FPREV),
                        ETA_LTP + ETA_LTD,
                        -ETA_LTD,
                    )
                nc.vector.scalar_tensor_tensor(
                    W[:, 0 : NOFF_A * CHUNK],
                    W[:, 0 : NOFF_A * CHUNK],
                    1.0 - WDECAY,
                    P[:, 0 : NOFF_A * CHUNK],
                    ALU.mult,
                    ALU.add,
                )
                nc.vector.tensor_scalar(
                    W[:, 0 : NOFF_A * CHUNK],
                    W[:, 0 : NOFF_A * CHUNK],
                    1.0,
                    0.0,
                    ALU.min,
                    ALU.max,
                )

            # ---- halo extraction (sync queue, gated on the collective) ----
            agf = agout.rearrange("p (y z) -> p y z", z=NZ)
            # left neighbor's top strip -> my bottom halo rows (ys 0,1)
            nc.sync.dma_start(
                f3(FOUT)[:, 0:2, 2 : 2 + NZ],
                agf[bass.ds(offL * NX, NX), 0:2],
            )
            # right neighbor's bottom strip -> my top halo rows (ys 10,11)
            nc.sync.dma_start(
                f3(FOUT)[:, 10:12, 2 : 2 + NZ],
                agf[bass.ds(offR * NX, NX), 2:4],
            )
            # patch halo rows of the +-1 x-shifted copies (only rows 1 and 10
            # are ever read: dy=+-1 slots with dx=+-1)
            for dx in (1, -1):
                xshift_rows(SO[dx], FOUT, dx, 0, 2)
                xshift_rows(SO[dx], FOUT, dx, 10, 12)

            # ---- STDP part B: dy != 0 slots (need the fresh halo) ----
            if t > 0:
                for k, d in enumerate(OFFSETS[NOFF_A:]):
                    kk = NOFF_A + k
                    nc.vector.affine_mul_reduce(
                        w3(P, kk),
                        ACC[:, kk : kk + 1],
                        fshift(FOUT, SO, d),
                        fint(FPREV),
                        ETA_LTP + ETA_LTD,
                        -ETA_LTD,
                    )
                nc.vector.scalar_tensor_tensor(
                    W[:, NOFF_A * CHUNK :],
                    W[:, NOFF_A * CHUNK :],
                    1.0 - WDECAY,
                    P[:, NOFF_A * CHUNK :],
                    ALU.mult,
                    ALU.add,
                )
                nc.vector.tensor_scalar(
                    W[:, NOFF_A * CHUNK :],
                    W[:, NOFF_A * CHUNK :],
                    1.0,
                    0.0,
                    ALU.min,
                    ALU.max,
                )

    nc.compile()
    return nc


def _shard_inputs(external_input, edge_values, edge_rows, edge_cols, nsteps):
    """Build per-core input maps (host-side sharding)."""
    ext = np.ascontiguousarray(np.asarray(external_input, dtype=np.float32))[:nsteps]
    vals = np.asarray(edge_values, dtype=np.float32)
    rows = np.asarray(edge_rows, dtype=np.int64)
    cols = np.asarray(edge_cols, dtype=np.int64)

    # dense weights keyed by destination: Wd[k, j] = w(edge j-d_k -> j)
    dlin = cols - rows
    offs_lin = np.array([d[0] * NY * NZ + d[1] * NZ + d[2] for d in OFFSETS])
    assert set(int(v) for v in np.unique(dlin)).issubset(
        set(int(v) for v in offs_lin)
    )
    k_of = np.zeros(int(offs_lin.max()) - int(offs_lin.min()) + 1, dtype=np.int64)
    for i, v in enumerate(offs_lin):
        k_of[int(v) - int(offs_lin.min())] = i
    ke = k_of[dlin - int(offs_lin.min())]
    Wd = np.zeros((NOFF, N), dtype=np.float32)
    Wd[ke, cols] = vals

    Wd = Wd.reshape(NOFF, NX, NY, NZ)
    ext = ext.reshape(nsteps, NX, NY, NZ)

    in_maps = []
    for c in range(NCORES):
        ylo = c * YS
        wc = np.ascontiguousarray(
            Wd[:, :, ylo : ylo + YS, :].transpose(1, 0, 2, 3)
        ).reshape(NX, NOFF * CHUNK)
        xc = np.ascontiguousarray(ext[:, :, ylo : ylo + YS, :]).reshape(
            nsteps, NX, CHUNK
        )
        msk = np.zeros((NX, 2), dtype=np.float32)
        msk[:, 0] = 0.0 if c == NCORES - 1 else 1.0  # top strip valid?
        msk[:, 1] = 0.0 if c == 0 else 1.0           # bottom strip valid?
        in_maps.append({"w0": wc, "xin": xc, "msk": msk})
    return in_maps


def kernel(external_input, edge_values, edge_rows, edge_cols, num_steps):
    nsteps = int(num_steps)
    if nsteps not in _CACHE:
        _CACHE[nsteps] = _build_graph(nsteps)
    nc = _CACHE[nsteps]

    in_maps = _shard_inputs(external_input, edge_values, edge_rows, edge_cols, nsteps)
    res = bass_utils.run_bass_kernel_spmd(
        nc,
        in_maps,
        core_ids=list(range(NCORES)),
        trace=bool(int(os.environ.get("BRAIN_TRACE", "0"))),
    )

    out = np.empty((nsteps, NX, NY, NZ), dtype=np.float32)
    for c in range(NCORES):
        ylo = c * YS
        out[:, :, ylo : ylo + YS, :] = res.results[c]["spk"].reshape(
            nsteps, NX, YS, NZ
        )
    kernel.last_results = res
    return out.reshape(nsteps, N)


# revision 5
# speedup vs baseline: 2.3405x; 1.4703x over previous
"""Trainium2 Bass kernel for Brain3DQTUNNetwork (gnn_message_passing).

The "sparse" graph is a fixed Manhattan-radius-2 stencil on a 64^3 grid
(24 offsets).  Weights are stored dense per offset slot, keyed by the
DESTINATION (col) index: W[k][j] = w(edge j-d_k -> j), 0 for invalid
edges.  The per-step segment_sum SpMV becomes 24 shifted elementwise
multiply-accumulates, and the STDP update becomes
    w = clip(w*(1-WDECAY) + prev * q_shift,  0, 1),   q(o) = 0.015*o - 0.005.
Invalid slots self-heal to 0 every step (q reads 0 / q(0) < 0 there and
the clip floors at 0).

Layout: 128 partitions = (y-half h, x): p = h*64 + x.  Each partition
holds a 4-y-plane sub-slab: field free axis = (y_sub + halo, z + pad) =
8*68 = 544.  y/z shifts are free-dim AP offsets; x shifts are 2
per-block SBUF->SBUF DMA partition-shifted copies; the h<->h halo is an
intra-core partition+-64 DMA copy.

Sharding: 8 cores x 8 y-planes.  All state stays SBUF-resident; per-step
cross-core traffic is one 8-rank AllGather of 2-row boundary strips.

Engine use: everything elementwise on Vector (DVE); sigmoid + masked
strip copies on Scalar; GpSimd only fires the collective.  The STDP
product uses the fused affine_mul_reduce DVE op (folds q() into the
multiply).  Offset slots are grouped so stencil taps whose windows
differ by a constant stride share one instruction (24 taps -> 12 ops),
with dy==0 groups first so half the STDP/W update overlaps the
AllGather latency.
"""

import os
import sys

sys.path.insert(0, "/opt/trn_rl_repo")

import numpy as np

import bass_rust
import concourse.bass as bass
import concourse.bacc as bacc
import concourse.mybir as mybir
import concourse.tile as tile
from concourse import bass_utils

# ---- problem constants (hardcoded; kernel.py must be self-contained) ----
GRID = (64, 64, 64)
NX, NY, NZ = GRID
N = NX * NY * NZ
RADIUS = 2
NCORES = 8
YS = NY // NCORES  # y-planes per core = 8

TAU = 20.0
REST_V = -65.0
EXC_THR = -50.0
INH_THR = -70.0
RESET_V = -65.0
ETA_LTP, ETA_LTD, WDECAY = 0.01, 0.005, 1e-05

# fp32-exact scalars matching the jax reference
DECAY = float(np.exp(np.float32(-1.0 / np.float32(TAU))).astype(np.float32))
ONE_MINUS_DECAY = float(np.float32(1.0) - np.float32(DECAY))
MIDPOINT = (EXC_THR + INH_THR) / 2.0  # -60.0

# ---- offset slot grouping ----
# Each group = (dx, [(dy, dz), ...]) where consecutive taps' field windows
# differ by a constant positive stride, so one strided AP covers the whole
# group.  Part A (dy == 0, k 0..11) needs no y-halo; part B (k 12..23) does.
GROUPS_A = [
    (0, [(0, 2), (0, 1)]),            # window offsets 136,137 (stride 1)
    (0, [(0, -1), (0, -2)]),          # 139,140
    (1, [(0, 1), (0, 0), (0, -1)]),   # 137,138,139
    (-1, [(0, 1), (0, 0), (0, -1)]),
    (2, [(0, 0)]),
    (-2, [(0, 0)]),
]
GROUPS_B = [
    (0, [(2, 0)]),                    # 2
    (0, [(1, 1), (1, 0), (1, -1)]),   # 69,70,71
    (0, [(-1, 1), (-1, 0), (-1, -1)]),  # 205,206,207
    (0, [(-2, 0)]),                   # 274
    (1, [(1, 0), (-1, 0)]),           # 70,206 (stride 136)
    (-1, [(1, 0), (-1, 0)]),
]
GROUPS = GROUPS_A + GROUPS_B
OFFSETS = [(dx, dy, dz) for dx, tap in GROUPS for dy, dz in tap]
NOFF = len(OFFSETS)  # 24
NOFF_A = sum(len(tap) for _, tap in GROUPS_A)  # 12
XSHIFTS = (1, -1, 2, -2)

# field geometry: partition p = h*64 + x, free = ys*68 + zs,
# ys = y_sub + 2 in [0,8), zs = z + 2 in [0,68)
PB = 2 * NX        # 128 partitions
SUB = YS // 2      # 4 own y rows per partition block
FYS = SUB + 4      # 8 field y rows (4 own + 2 halo each side)
FZS = NZ + 4       # 68 field z cols
FFREE = FYS * FZS  # 544
CHUNK = SUB * NZ   # 256 elems per partition for compact tiles
F32 = mybir.dt.float32

_CACHE = {}


def _build_graph(nsteps):
    nc = bacc.Bacc(
        "TRN2",
        target_bir_lowering=False,
        debug=False,
        enable_asserts=True,
        num_devices=NCORES,
    )
    w0_d = nc.dram_tensor("w0", [PB, NOFF * CHUNK], F32, kind="ExternalInput").ap()
    xin_d = nc.dram_tensor("xin", [nsteps, PB, CHUNK], F32, kind="ExternalInput").ap()
    msk_d = nc.dram_tensor("msk", [PB, 1], F32, kind="ExternalInput").ap()
    spk_d = nc.dram_tensor("spk", [nsteps, PB, CHUNK], F32, kind="ExternalOutput").ap()

    AT = mybir.ActivationFunctionType
    ALU = mybir.AluOpType

    with tile.TileContext(nc) as tc, tc.tile_pool(
        name="state", bufs=1
    ) as st, tc.tile_pool(name="dram", bufs=1, space="DRAM") as dr:
        # persistent state tiles (distinct names -> distinct slots)
        W = st.tile([PB, NOFF * CHUNK], F32, name="W")
        P = st.tile([PB, NOFF * CHUNK], F32, name="P")
        FA = st.tile([PB, FFREE], F32, name="FA")
        FB = st.tile([PB, FFREE], F32, name="FB")
        SO = {dx: st.tile([PB, FFREE], F32, name=f"SOx{dx+2}") for dx in XSHIFTS}
        V = st.tile([PB, CHUNK], F32, name="V")
        SYN = st.tile([PB, CHUNK], F32, name="SYN")
        S = st.tile([PB, CHUNK], F32, name="S")
        II = st.tile([PB, CHUNK], F32, name="II")
        G = st.tile([PB, CHUNK], F32, name="G")
        NN = st.tile([PB, CHUNK], F32, name="NN")
        T1 = st.tile([PB, CHUNK], F32, name="T1")
        T2 = st.tile([PB, CHUNK], F32, name="T2")
        XIP = st.tile([PB, CHUNK], F32, name="XIP")
        MSK = st.tile([PB, 1], F32, name="MSK")
        SND = st.tile([PB, 2 * NZ], F32, name="SND")
        B30 = st.tile([PB, 1], F32, name="B30")  # sigmoid bias const
        ACC = st.tile([PB, 2 * len(GROUPS)], F32, name="ACC")  # amr scratch

        def f3(t):  # [PB, FFREE] -> [PB, FYS, FZS]
            return t.rearrange("p (y z) -> p y z", z=FZS)

        def c3(t):  # [PB, CHUNK] -> [PB, SUB, NZ]
            return t.rearrange("p (y z) -> p y z", z=NZ)

        def wg(t, k0, m):  # slots k0..k0+m of W/P -> [PB, m, SUB, NZ]
            return t.rearrange("p (k y z) -> p k y z", k=NOFF, z=NZ)[:, k0 : k0 + m]

        def fint(t):  # own interior of a field tile -> [PB, SUB, NZ]
            return f3(t)[:, 2 : 2 + SUB, 2 : 2 + NZ]

        def gwin(base, xs, g):
            # grouped source view: one AP covering every tap in group g,
            # group axis strided by the constant window-offset delta
            dx, taps = g
            t = base if dx == 0 else xs[dx]
            if len(taps) == 1:
                dy, dz = taps[0]
                return f3(t)[:, 2 - dy : 2 - dy + SUB, 2 - dz : 2 - dz + NZ]
            offs = [(2 - dy) * FZS + (2 - dz) for dy, dz in taps]
            delta = offs[1] - offs[0]
            assert delta > 0 and all(
                offs[i + 1] - offs[i] == delta for i in range(len(offs) - 1)
            )
            full = t[:]
            return bass_rust.AP(
                tensor=full.tensor,
                offset=offs[0],
                ap=[[FFREE, PB], [delta, len(taps)], [FZS, SUB], [1, NZ]],
            )

        def xshift_blk(dst, src, dx, h, r0, r1):
            # dst[p, r0:r1, :] = src[p - dx, r0:r1, :] within x-block h
            base = h * NX
            a, b = base + max(0, dx), base + NX + min(0, dx)
            nc.sync.dma_start(
                f3(dst)[a:b, r0:r1, :], f3(src)[a - dx : b - dx, r0:r1, :]
            )

        # ---- init ----
        nc.vector.memset(FA[:], 0.0)
        nc.vector.memset(FB[:], 0.0)
        for dx in XSHIFTS:
            nc.vector.memset(SO[dx][:], 0.0)
        nc.vector.memset(V[:], REST_V)
        nc.vector.memset(B30[:], -0.5 * MIDPOINT)
        nc.sync.dma_start(W[:], w0_d[:])
        nc.sync.dma_start(MSK[:], msk_d[:])

        pid = nc.sync.partition_id()
        # strip row indices into the gathered [8*PB, ...] buffer, in units
        # of 64 partitions: left neighbor's top strip / right's bottom strip
        selL = nc.sync.snap(
            ((pid + NCORES - 1) % NCORES) * 2 + 1, min_val=0, max_val=2 * NCORES - 1
        )
        selR = nc.sync.snap(
            ((pid + 1) % NCORES) * 2, min_val=0, max_val=2 * NCORES - 2
        )

        fields = [FA, FB]
        for t in range(nsteps):
            FPREV = fields[t % 2]       # holds out_{t-1} (with halos)
            FOUT = fields[(t + 1) % 2]  # will hold out_t

            # external input, prescaled by (1-decay) (scalar engine, off-path)
            nc.sync.dma_start(XIP[:], xin_d[t])
            nc.scalar.mul(XIP[:], XIP[:], ONE_MINUS_DECAY)

            # ---- syn: grouped shifted products then pairwise tree sum ----
            k0 = 0
            for g in GROUPS:
                m = len(g[1])
                nc.vector.tensor_tensor(
                    wg(P, k0, m), wg(W, k0, m), gwin(FPREV, SO, g), ALU.mult
                )
                k0 += m
            # tree-reduce the 24 slots (in place over P)
            Pf = P.rearrange("p (k f) -> p k f", k=NOFF)
            nslots = NOFF
            while nslots > 1:
                half = nslots // 2
                nc.vector.tensor_tensor(
                    Pf[:, 0:half],
                    Pf[:, 0:half],
                    Pf[:, half : 2 * half],
                    ALU.add,
                )
                if nslots % 2:
                    nc.vector.tensor_tensor(
                        Pf[:, 0], Pf[:, 0], Pf[:, nslots - 1], ALU.add
                    )
                nslots = half
            # SYN = (P0 * (1-decay)) + XIP ; then v = v*decay + SYN
            nc.vector.scalar_tensor_tensor(
                SYN[:], Pf[:, 0], ONE_MINUS_DECAY, XIP[:], ALU.mult, ALU.add
            )
            nc.vector.scalar_tensor_tensor(
                V[:], V[:], DECAY, SYN[:], ALU.mult, ALU.add
            )

            # ---- neuron update (vector TS + scalar sigmoid) ----
            nc.scalar.activation(G[:], V[:], AT.Sigmoid, bias=B30[:, 0:1], scale=0.5)
            nc.vector.tensor_single_scalar(S[:], V[:], EXC_THR, ALU.is_ge)
            nc.vector.tensor_single_scalar(II[:], V[:], INH_THR, ALU.is_le)
            nc.vector.tensor_tensor(NN[:], S[:], II[:], ALU.add)
            nc.vector.scalar_tensor_tensor(
                T1[:], NN[:], 1.0, G[:], ALU.subtract, ALU.mult
            )
            # out = s - (n-1)*g, written straight into the FOUT interior
            nc.vector.tensor_tensor(fint(FOUT), c3(S), c3(T1), ALU.subtract)

            # ---- intra-core h<->h halo + boundary strips: launch ASAP ----
            # block 0's top halo (ys 6:8) = block 1's own rows 2:4 (p+64);
            # block 1's bottom halo (ys 0:2) = block 0's own rows 4:6 (p-64)
            nc.sync.dma_start(f3(FOUT)[0:NX, 6:8, :], f3(FOUT)[NX:PB, 2:4, :])
            nc.sync.dma_start(f3(FOUT)[NX:PB, 0:2, :], f3(FOUT)[0:NX, 4:6, :])
            # strips: p<64 -> core's bottom 2 rows (ys 2:4 of block 0),
            #         p>=64 -> core's top 2 rows (ys 4:6 of block 1), masked
            snd3 = SND.rearrange("p (y z) -> p y z", z=NZ)
            nc.scalar.activation(
                snd3[0:NX], f3(FOUT)[0:NX, 2:4, 2 : 2 + NZ],
                AT.Copy, bias=0.0, scale=MSK[0:NX, 0:1],
            )
            nc.scalar.activation(
                snd3[NX:PB], f3(FOUT)[NX:PB, 4:6, 2 : 2 + NZ],
                AT.Copy, bias=0.0, scale=MSK[NX:PB, 0:1],
            )
            agin = dr.tile([PB, 2 * NZ], F32, name=f"agin{t}")
            agout = dr.tile(
                [NCORES * PB, 2 * NZ], F32, addr_space="Shared", name=f"agout{t}"
            )
            nc.sync.dma_start(agin[:], SND[:])
            nc.gpsimd.collective_compute(
                "AllGather",
                ALU.bypass,
                replica_groups=[list(range(NCORES))],
                ins=[agin.opt()],
                outs=[agout.opt()],
            )

            # ---- off the critical path while the collective flies ----
            # v reset where spiked: v -= (v - RESET_V) * s
            nc.vector.scalar_tensor_tensor(
                T2[:], V[:], -RESET_V, S[:], ALU.add, ALU.mult
            )
            nc.vector.tensor_tensor(V[:], V[:], T2[:], ALU.subtract)
            # spike train output for this step
            nc.sync.dma_start(spk_d[t], S[:])
            # x-shifted copies of out_t over the rows valid pre-exchange
            # (block 0 has ys 2:8 after the intra copy, block 1 has ys 0:6;
            #  SO[+-1] is read at ys 1:7, SO[+-2] at ys 2:6)
            for dx in (1, -1):
                xshift_blk(SO[dx], FOUT, dx, 0, 2, 7)
                xshift_blk(SO[dx], FOUT, dx, 1, 1, 6)
            for dx in (2, -2):
                xshift_blk(SO[dx], FOUT, dx, 0, 2, 6)
                xshift_blk(SO[dx], FOUT, dx, 1, 2, 6)

            # ---- STDP part A: dy == 0 groups need no y-halo (t=0 skips) ----
            # dw[k][j] = prev[j] * (0.015*out[j-dk] - 0.005) via fused DVE op
            if t > 0:
                for k, d in enumerate(OFFSETS[:NOFF_A]):
                    dxk, dy, dz = d
                    src = FOUT if dxk == 0 else SO[dxk]
                    nc.vector.affine_mul_reduce(
                        wg(P, k, 1).squeeze(1),
                        ACC[:, k : k + 1],
                        f3(src)[:, 2 - dy : 2 - dy + SUB, 2 - dz : 2 - dz + NZ],
                        fint(FPREV),
                        ETA_LTP + ETA_LTD,
                        -ETA_LTD,
                    )
                nc.vector.scalar_tensor_tensor(
                    W[:, 0 : NOFF_A * CHUNK],
                    W[:, 0 : NOFF_A * CHUNK],
                    1.0 - WDECAY,
                    P[:, 0 : NOFF_A * CHUNK],
                    ALU.mult,
                    ALU.add,
                )
                nc.vector.tensor_scalar(
                    W[:, 0 : NOFF_A * CHUNK],
                    W[:, 0 : NOFF_A * CHUNK],
                    1.0,
                    0.0,
                    ALU.min,
                    ALU.max,
                )

            # ---- halo extraction (sync queue, gated on the collective) ----
            agf = agout.rearrange("p (y z) -> p y z", z=NZ)
            # left neighbor's top strip -> block 0's bottom halo (ys 0:2)
            nc.sync.dma_start(
                f3(FOUT)[0:NX, 0:2, 2 : 2 + NZ],
                agf[bass.ds(selL * NX, NX)],
            )
            # right neighbor's bottom strip -> block 1's top halo (ys 6:8)
            nc.sync.dma_start(
                f3(FOUT)[NX:PB, 6:8, 2 : 2 + NZ],
                agf[bass.ds(selR * NX, NX)],
            )
            # patch the x-shift rows that waited on the exchange
            for dx in (1, -1):
                xshift_blk(SO[dx], FOUT, dx, 0, 1, 2)
                xshift_blk(SO[dx], FOUT, dx, 1, 6, 7)

            # ---- STDP part B: dy != 0 groups (need the fresh halo) ----
            if t > 0:
                for k, d in enumerate(OFFSETS[NOFF_A:], start=NOFF_A):
                    dxk, dy, dz = d
                    src = FOUT if dxk == 0 else SO[dxk]
                    nc.vector.affine_mul_reduce(
                        wg(P, k, 1).squeeze(1),
                        ACC[:, k : k + 1],
                        f3(src)[:, 2 - dy : 2 - dy + SUB, 2 - dz : 2 - dz + NZ],
                        fint(FPREV),
                        ETA_LTP + ETA_LTD,
                        -ETA_LTD,
                    )
                nc.vector.scalar_tensor_tensor(
                    W[:, NOFF_A * CHUNK :],
                    W[:, NOFF_A * CHUNK :],
                    1.0 - WDECAY,
                    P[:, NOFF_A * CHUNK :],
                    ALU.mult,
                    ALU.add,
                )
                nc.vector.tensor_scalar(
                    W[:, NOFF_A * CHUNK :],
                    W[:, NOFF_A * CHUNK :],
                    1.0,
                    0.0,
                    ALU.min,
                    ALU.max,
                )

    nc.compile()
    return nc


def _shard_inputs(external_input, edge_values, edge_rows, edge_cols, nsteps):
    """Build per-core input maps (host-side sharding)."""
    ext = np.ascontiguousarray(np.asarray(external_input, dtype=np.float32))[:nsteps]
    vals = np.asarray(edge_values, dtype=np.float32)
    rows = np.asarray(edge_rows, dtype=np.int64)
    cols = np.asarray(edge_cols, dtype=np.int64)

    # dense weights keyed by destination: Wd[k, j] = w(edge j-d_k -> j)
    dlin = cols - rows
    offs_lin = np.array([d[0] * NY * NZ + d[1] * NZ + d[2] for d in OFFSETS])
    assert set(int(v) for v in np.unique(dlin)).issubset(
        set(int(v) for v in offs_lin)
    )
    k_of = np.zeros(int(offs_lin.max()) - int(offs_lin.min()) + 1, dtype=np.int64)
    for i, v in enumerate(offs_lin):
        k_of[int(v) - int(offs_lin.min())] = i
    ke = k_of[dlin - int(offs_lin.min())]
    Wd = np.zeros((NOFF, N), dtype=np.float32)
    Wd[ke, cols] = vals

    # [NOFF, NX, NCORES, 2(h), SUB, NZ]
    Wd = Wd.reshape(NOFF, NX, NCORES, 2, SUB, NZ)
    ext = ext.reshape(nsteps, NX, NCORES, 2, SUB, NZ)

    in_maps = []
    for c in range(NCORES):
        # partition p = h*64 + x
        wc = np.ascontiguousarray(
            Wd[:, :, c].transpose(2, 1, 0, 3, 4)
        ).reshape(PB, NOFF * CHUNK)
        xc = np.ascontiguousarray(
            ext[:, :, c].transpose(0, 2, 1, 3, 4)
        ).reshape(nsteps, PB, CHUNK)
        msk = np.zeros((PB, 1), dtype=np.float32)
        msk[0:NX, 0] = 0.0 if c == 0 else 1.0           # bottom strip valid?
        msk[NX:PB, 0] = 0.0 if c == NCORES - 1 else 1.0  # top strip valid?
        in_maps.append({"w0": wc, "xin": xc, "msk": msk})
    return in_maps


def kernel(external_input, edge_values, edge_rows, edge_cols, num_steps):
    nsteps = int(num_steps)
    if nsteps not in _CACHE:
        _CACHE[nsteps] = _build_graph(nsteps)
    nc = _CACHE[nsteps]

    in_maps = _shard_inputs(external_input, edge_values, edge_rows, edge_cols, nsteps)
    res = bass_utils.run_bass_kernel_spmd(
        nc,
        in_maps,
        core_ids=list(range(NCORES)),
        trace=bool(int(os.environ.get("BRAIN_TRACE", "0"))),
    )

    out = np.empty((nsteps, NX, NCORES, 2, SUB, NZ), dtype=np.float32)
    for c in range(NCORES):
        out[:, :, c] = (
            res.results[c]["spk"]
            .reshape(nsteps, 2, NX, SUB, NZ)
            .transpose(0, 2, 1, 3, 4)
        )
    kernel.last_results = res
    return out.reshape(nsteps, N)


# revision 14
# speedup vs baseline: 2.5191x; 1.0763x over previous
"""Trainium2 Bass kernel for Brain3DQTUNNetwork (gnn_message_passing).

The "sparse" graph is a fixed Manhattan-radius-2 stencil on a 64^3 grid
(24 offsets).  Weights are stored dense per offset slot, keyed by the
DESTINATION (col) index: W[k][j] = w(edge j-d_k -> j), 0 for invalid
edges.  The per-step segment_sum SpMV becomes 24 shifted elementwise
multiply-accumulates, and the STDP update becomes
    w = clip(w*(1-WDECAY) + prev * q_shift,  0, 1),   q(o) = 0.015*o - 0.005.
Invalid slots self-heal to 0 every step (q reads 0 / q(0) < 0 there and
the clip floors at 0).

Layout: 128 partitions = (y-half h, x): p = h*64 + x.  Each partition
holds a 4-y-plane sub-slab: field free axis = (y_sub + halo, z + pad) =
8*68 = 544.  y/z shifts are free-dim AP offsets; x shifts are 2
per-block SBUF->SBUF DMA partition-shifted copies; the h<->h halo is an
intra-core partition+-64 DMA copy.

Sharding: 8 cores x 8 y-planes.  All state stays SBUF-resident; per-step
cross-core traffic is one 8-rank AllGather of 2-row boundary strips.

Engine use: everything elementwise on Vector (DVE); sigmoid + masked
strip copies on Scalar; GpSimd only fires the collective.  The STDP
product uses the fused affine_mul_reduce DVE op (folds q() into the
multiply).  Offset slots are grouped so stencil taps whose windows
differ by a constant stride share one instruction (24 taps -> 12 ops),
with dy==0 groups first so half the STDP/W update overlaps the
AllGather latency.
"""

import os
import sys

sys.path.insert(0, "/opt/trn_rl_repo")

import numpy as np

import bass_rust
import concourse.bass as bass
import concourse.bacc as bacc
import concourse.mybir as mybir
import concourse.tile as tile
from concourse import bass_utils

# ---- problem constants (hardcoded; kernel.py must be self-contained) ----
GRID = (64, 64, 64)
NX, NY, NZ = GRID
N = NX * NY * NZ
RADIUS = 2
NCORES = 8
YS = NY // NCORES  # y-planes per core = 8

TAU = 20.0
REST_V = -65.0
EXC_THR = -50.0
INH_THR = -70.0
RESET_V = -65.0
ETA_LTP, ETA_LTD, WDECAY = 0.01, 0.005, 1e-05

# fp32-exact scalars matching the jax reference
DECAY = float(np.exp(np.float32(-1.0 / np.float32(TAU))).astype(np.float32))
ONE_MINUS_DECAY = float(np.float32(1.0) - np.float32(DECAY))
MIDPOINT = (EXC_THR + INH_THR) / 2.0  # -60.0

# ---- offset slot grouping ----
# Each group = (dx, [(dy, dz), ...]) where consecutive taps' field windows
# differ by a constant positive stride, so one strided AP covers the whole
# group.  Part A (dy == 0, k 0..11) needs no y-halo; part B (k 12..23) does.
GROUPS_A = [
    (0, [(0, 2), (0, 1)]),            # window offsets 136,137 (stride 1)
    (0, [(0, -1), (0, -2)]),          # 139,140
    (1, [(0, 1), (0, 0), (0, -1)]),   # 137,138,139
    (-1, [(0, 1), (0, 0), (0, -1)]),
    (2, [(0, 0)]),
    (-2, [(0, 0)]),
]
GROUPS_B = [
    (0, [(2, 0)]),                    # 2
    (0, [(1, 1), (1, 0), (1, -1)]),   # 69,70,71
    (0, [(-1, 1), (-1, 0), (-1, -1)]),  # 205,206,207
    (0, [(-2, 0)]),                   # 274
    (1, [(1, 0), (-1, 0)]),           # 70,206 (stride 136)
    (-1, [(1, 0), (-1, 0)]),
]
GROUPS = GROUPS_A + GROUPS_B
OFFSETS = [(dx, dy, dz) for dx, tap in GROUPS for dy, dz in tap]
NOFF = len(OFFSETS)  # 24
NOFF_A = sum(len(tap) for _, tap in GROUPS_A)  # 12
XSHIFTS = (1, -1, 2, -2)

# field geometry: partition p = h*64 + x, free = ys*68 + zs,
# ys = y_sub + 2 in [0,8), zs = z + 2 in [0,68)
PB = 2 * NX        # 128 partitions
SUB = YS // 2      # 4 own y rows per partition block
FYS = SUB + 4      # 8 field y rows (4 own + 2 halo each side)
FZS = NZ + 4       # 68 field z cols
FFREE = FYS * FZS  # 544
CHUNK = SUB * NZ   # 256 elems per partition for compact tiles
F32 = mybir.dt.float32

_CACHE = {}


def _build_graph(nsteps):
    nc = bacc.Bacc(
        "TRN2",
        target_bir_lowering=False,
        debug=False,
        enable_asserts=True,
        num_devices=NCORES,
    )
    w0_d = nc.dram_tensor("w0", [PB, NOFF * CHUNK], F32, kind="ExternalInput").ap()
    xin_d = nc.dram_tensor("xin", [nsteps, PB, CHUNK], F32, kind="ExternalInput").ap()
    msk_d = nc.dram_tensor("msk", [PB, 1], F32, kind="ExternalInput").ap()
    spk_d = nc.dram_tensor("spk", [nsteps, PB, CHUNK], F32, kind="ExternalOutput").ap()

    AT = mybir.ActivationFunctionType
    ALU = mybir.AluOpType

    with tile.TileContext(nc) as tc, tc.tile_pool(
        name="state", bufs=1
    ) as st, tc.tile_pool(name="dram", bufs=1, space="DRAM") as dr:
        # persistent state tiles (distinct names -> distinct slots)
        W = st.tile([PB, NOFF * CHUNK], F32, name="W")
        P = st.tile([PB, NOFF * CHUNK], F32, name="P")
        FA = st.tile([PB, FFREE], F32, name="FA")
        FB = st.tile([PB, FFREE], F32, name="FB")
        SO = {dx: st.tile([PB, FFREE], F32, name=f"SOx{dx+2}") for dx in XSHIFTS}
        V = st.tile([PB, CHUNK], F32, name="V")
        SYN = st.tile([PB, CHUNK], F32, name="SYN")
        S = st.tile([PB, CHUNK], F32, name="S")
        II = st.tile([PB, CHUNK], F32, name="II")
        G = st.tile([PB, CHUNK], F32, name="G")
        NN = st.tile([PB, CHUNK], F32, name="NN")
        T1 = st.tile([PB, CHUNK], F32, name="T1")
        T2 = st.tile([PB, CHUNK], F32, name="T2")
        XIP = st.tile([PB, CHUNK], F32, name="XIP")
        MSK = st.tile([PB, 1], F32, name="MSK")
        SND = st.tile([PB, 2 * NZ], F32, name="SND")
        B30 = st.tile([PB, 1], F32, name="B30")  # sigmoid bias const
        ACC = st.tile([PB, 2 * len(GROUPS)], F32, name="ACC")  # amr scratch

        def f3(t):  # [PB, FFREE] -> [PB, FYS, FZS]
            return t.rearrange("p (y z) -> p y z", z=FZS)

        def c3(t):  # [PB, CHUNK] -> [PB, SUB, NZ]
            return t.rearrange("p (y z) -> p y z", z=NZ)

        def wg(t, k0, m):  # slots k0..k0+m of W/P -> [PB, m, SUB, NZ]
            return t.rearrange("p (k y z) -> p k y z", k=NOFF, z=NZ)[:, k0 : k0 + m]

        def fint(t):  # own interior of a field tile -> [PB, SUB, NZ]
            return f3(t)[:, 2 : 2 + SUB, 2 : 2 + NZ]

        def xshift(eng, dst, src, dx, rows):
            # dst[p, rows, :] = src[p - dx, rows, :] within each x-block
            # (one DMA per block; DMA has no partition-alignment limits)
            for h in (0, 1):
                base = h * NX
                a, b = base + max(0, dx), base + NX + min(0, dx)
                eng.dma_start(
                    f3(dst)[a:b, rows, :], f3(src)[a - dx : b - dx, rows, :]
                )

        def gwin(base, xs, g):
            # grouped source view: one AP covering every tap in group g,
            # group axis strided by the constant window-offset delta
            dx, taps = g
            t = base if dx == 0 else xs[dx]
            if len(taps) == 1:
                dy, dz = taps[0]
                return f3(t)[:, 2 - dy : 2 - dy + SUB, 2 - dz : 2 - dz + NZ]
            offs = [(2 - dy) * FZS + (2 - dz) for dy, dz in taps]
            delta = offs[1] - offs[0]
            assert delta > 0 and all(
                offs[i + 1] - offs[i] == delta for i in range(len(offs) - 1)
            )
            full = t[:]
            return bass_rust.AP(
                tensor=full.tensor,
                offset=offs[0],
                ap=[[FFREE, PB], [delta, len(taps)], [FZS, SUB], [1, NZ]],
            )

        # ---- init ----
        nc.vector.memset(FA[:], 0.0)
        nc.vector.memset(FB[:], 0.0)
        for dx in XSHIFTS:
            nc.vector.memset(SO[dx][:], 0.0)
        nc.vector.memset(V[:], REST_V)
        nc.vector.memset(B30[:], -0.5 * MIDPOINT)
        nc.sync.dma_start(W[:], w0_d[:])
        nc.sync.dma_start(MSK[:], msk_d[:])

        pid = nc.sync.partition_id()
        # strip row indices into the gathered [8*PB, ...] buffer, in units
        # of 64 partitions: left neighbor's top strip / right's bottom strip
        selL = nc.sync.snap(
            ((pid + NCORES - 1) % NCORES) * 2 + 1, min_val=0, max_val=2 * NCORES - 1
        )
        selR = nc.sync.snap(
            ((pid + 1) % NCORES) * 2, min_val=0, max_val=2 * NCORES - 2
        )

        fields = [FA, FB]
        for t in range(nsteps):
            FPREV = fields[t % 2]       # holds out_{t-1} (with halos)
            FOUT = fields[(t + 1) % 2]  # will hold out_t

            # external input, prescaled by (1-decay) (scalar engine, off-path)
            nc.sync.dma_start(XIP[:], xin_d[t])
            nc.scalar.mul(XIP[:], XIP[:], ONE_MINUS_DECAY)

            # ---- syn: grouped shifted products then pairwise tree sum ----
            k0 = 0
            for g in GROUPS:
                m = len(g[1])
                nc.vector.tensor_tensor(
                    wg(P, k0, m), wg(W, k0, m), gwin(FPREV, SO, g), ALU.mult
                )
                k0 += m
            # tree-reduce the 24 slots (in place over P)
            Pf = P.rearrange("p (k f) -> p k f", k=NOFF)
            nslots = NOFF
            while nslots > 1:
                half = nslots // 2
                nc.vector.tensor_tensor(
                    Pf[:, 0:half],
                    Pf[:, 0:half],
                    Pf[:, half : 2 * half],
                    ALU.add,
                )
                if nslots % 2:
                    nc.vector.tensor_tensor(
                        Pf[:, 0], Pf[:, 0], Pf[:, nslots - 1], ALU.add
                    )
                nslots = half
            # SYN = (P0 * (1-decay)) + XIP ; then v = v*decay + SYN
            nc.vector.scalar_tensor_tensor(
                SYN[:], Pf[:, 0], ONE_MINUS_DECAY, XIP[:], ALU.mult, ALU.add
            )
            nc.vector.scalar_tensor_tensor(
                V[:], V[:], DECAY, SYN[:], ALU.mult, ALU.add
            )

            # ---- neuron update (vector TS + scalar sigmoid) ----
            nc.scalar.activation(G[:], V[:], AT.Sigmoid, bias=B30[:, 0:1], scale=0.5)
            nc.vector.tensor_single_scalar(S[:], V[:], EXC_THR, ALU.is_ge)
            # spike train output for this step (early: S is final here)
            nc.sync.dma_start(spk_d[t], S[:])
            nc.vector.tensor_single_scalar(II[:], V[:], INH_THR, ALU.is_le)
            nc.vector.tensor_tensor(NN[:], S[:], II[:], ALU.add)
            nc.vector.scalar_tensor_tensor(
                T1[:], NN[:], 1.0, G[:], ALU.subtract, ALU.mult
            )
            # out = s - (n-1)*g, written straight into the FOUT interior
            nc.vector.tensor_tensor(fint(FOUT), c3(S), c3(T1), ALU.subtract)

            # ---- intra-core h<->h halo + boundary strips: launch ASAP ----
            # block 0's top halo (ys 6:8) = block 1's own rows 2:4 (p+64);
            # block 1's bottom halo (ys 0:2) = block 0's own rows 4:6 (p-64)
            nc.sync.dma_start(f3(FOUT)[0:NX, 6:8, :], f3(FOUT)[NX:PB, 2:4, :])
            nc.sync.dma_start(f3(FOUT)[NX:PB, 0:2, :], f3(FOUT)[0:NX, 4:6, :])
            # strips: p<64 -> core's bottom 2 rows (ys 2:4 of block 0),
            #         p>=64 -> core's top 2 rows (ys 4:6 of block 1), masked
            snd3 = SND.rearrange("p (y z) -> p y z", z=NZ)
            nc.scalar.activation(
                snd3[0:NX], f3(FOUT)[0:NX, 2:4, 2 : 2 + NZ],
                AT.Copy, bias=0.0, scale=MSK[0:NX, 0:1],
            )
            nc.scalar.activation(
                snd3[NX:PB], f3(FOUT)[NX:PB, 4:6, 2 : 2 + NZ],
                AT.Copy, bias=0.0, scale=MSK[NX:PB, 0:1],
            )
            agin = dr.tile([PB, 2 * NZ], F32, name=f"agin{t}")
            agout = dr.tile(
                [NCORES * PB, 2 * NZ], F32, addr_space="Shared", name=f"agout{t}"
            )
            nc.sync.dma_start(agin[:], SND[:])
            nc.gpsimd.collective_compute(
                "AllGather",
                ALU.bypass,
                replica_groups=[list(range(NCORES))],
                ins=[agin.opt()],
                outs=[agout.opt()],
            )

            # ---- off the critical path while the collective flies ----
            # urgent x-shifted copies of out_t: interior rows 2:6 only (all
            # that part A and the next step's dy==0 syn taps read), one DMA
            # per dx, posted from otherwise-idle engine queues in parallel
            xshift(nc.scalar, SO[1], FOUT, 1, slice(2, 6))
            xshift(nc.scalar, SO[-1], FOUT, -1, slice(2, 6))
            xshift(nc.sync, SO[2], FOUT, 2, slice(2, 6))
            xshift(nc.sync, SO[-2], FOUT, -2, slice(2, 6))
            # v reset where spiked: v -= (v - RESET_V) * s
            nc.vector.scalar_tensor_tensor(
                T2[:], V[:], -RESET_V, S[:], ALU.add, ALU.mult
            )
            nc.vector.tensor_tensor(V[:], V[:], T2[:], ALU.subtract)

            # ---- STDP part A: dy == 0 groups need no y-halo (t=0 skips) ----
            # dw[k][j] = prev[j] * (0.015*out[j-dk] - 0.005) via fused DVE op
            if t > 0:
                for k, d in enumerate(OFFSETS[:NOFF_A]):
                    dxk, dy, dz = d
                    src = FOUT if dxk == 0 else SO[dxk]
                    nc.vector.affine_mul_reduce(
                        wg(P, k, 1).squeeze(1),
                        ACC[:, k : k + 1],
                        f3(src)[:, 2 - dy : 2 - dy + SUB, 2 - dz : 2 - dz + NZ],
                        fint(FPREV),
                        ETA_LTP + ETA_LTD,
                        -ETA_LTD,
                    )
                nc.vector.scalar_tensor_tensor(
                    W[:, 0 : NOFF_A * CHUNK],
                    W[:, 0 : NOFF_A * CHUNK],
                    1.0 - WDECAY,
                    P[:, 0 : NOFF_A * CHUNK],
                    ALU.mult,
                    ALU.add,
                )
                nc.vector.tensor_scalar(
                    W[:, 0 : NOFF_A * CHUNK],
                    W[:, 0 : NOFF_A * CHUNK],
                    1.0,
                    0.0,
                    ALU.min,
                    ALU.max,
                )

            # ---- halo extraction (sync queue, gated on the collective) ----
            agf = agout.rearrange("p (y z) -> p y z", z=NZ)
            # left neighbor's top strip -> block 0's bottom halo (ys 0:2)
            nc.sync.dma_start(
                f3(FOUT)[0:NX, 0:2, 2 : 2 + NZ],
                agf[bass.ds(selL * NX, NX)],
            )
            # right neighbor's bottom strip -> block 1's top halo (ys 6:8)
            nc.sync.dma_start(
                f3(FOUT)[NX:PB, 6:8, 2 : 2 + NZ],
                agf[bass.ds(selR * NX, NX)],
            )
            # patch the x-shift halo rows 1 and 6 (read only by dy=+-1 taps
            # with dx=+-1); row 1 needs the exchange in block 0 and the intra
            # copy in block 1, row 6 vice versa
            xshift(nc.sync, SO[1], FOUT, 1, slice(1, 7, 5))
            xshift(nc.scalar, SO[-1], FOUT, -1, slice(1, 7, 5))

            # ---- STDP part B: dy != 0 groups (need the fresh halo) ----
            if t > 0:
                for k, d in enumerate(OFFSETS[NOFF_A:], start=NOFF_A):
                    dxk, dy, dz = d
                    src = FOUT if dxk == 0 else SO[dxk]
                    nc.vector.affine_mul_reduce(
                        wg(P, k, 1).squeeze(1),
                        ACC[:, k : k + 1],
                        f3(src)[:, 2 - dy : 2 - dy + SUB, 2 - dz : 2 - dz + NZ],
                        fint(FPREV),
                        ETA_LTP + ETA_LTD,
                        -ETA_LTD,
                    )
                nc.vector.scalar_tensor_tensor(
                    W[:, NOFF_A * CHUNK :],
                    W[:, NOFF_A * CHUNK :],
                    1.0 - WDECAY,
                    P[:, NOFF_A * CHUNK :],
                    ALU.mult,
                    ALU.add,
                )
                nc.vector.tensor_scalar(
                    W[:, NOFF_A * CHUNK :],
                    W[:, NOFF_A * CHUNK :],
                    1.0,
                    0.0,
                    ALU.min,
                    ALU.max,
                )

    nc.compile()
    return nc


def _shard_inputs(external_input, edge_values, edge_rows, edge_cols, nsteps):
    """Build per-core input maps (host-side sharding)."""
    ext = np.ascontiguousarray(np.asarray(external_input, dtype=np.float32))[:nsteps]
    vals = np.asarray(edge_values, dtype=np.float32)
    rows = np.asarray(edge_rows, dtype=np.int64)
    cols = np.asarray(edge_cols, dtype=np.int64)

    # dense weights keyed by destination: Wd[k, j] = w(edge j-d_k -> j)
    dlin = cols - rows
    offs_lin = np.array([d[0] * NY * NZ + d[1] * NZ + d[2] for d in OFFSETS])
    assert set(int(v) for v in np.unique(dlin)).issubset(
        set(int(v) for v in offs_lin)
    )
    k_of = np.zeros(int(offs_lin.max()) - int(offs_lin.min()) + 1, dtype=np.int64)
    for i, v in enumerate(offs_lin):
        k_of[int(v) - int(offs_lin.min())] = i
    ke = k_of[dlin - int(offs_lin.min())]
    Wd = np.zeros((NOFF, N), dtype=np.float32)
    Wd[ke, cols] = vals

    # [NOFF, NX, NCORES, 2(h), SUB, NZ]
    Wd = Wd.reshape(NOFF, NX, NCORES, 2, SUB, NZ)
    ext = ext.reshape(nsteps, NX, NCORES, 2, SUB, NZ)

    in_maps = []
    for c in range(NCORES):
        # partition p = h*64 + x
        wc = np.ascontiguousarray(
            Wd[:, :, c].transpose(2, 1, 0, 3, 4)
        ).reshape(PB, NOFF * CHUNK)
        xc = np.ascontiguousarray(
            ext[:, :, c].transpose(0, 2, 1, 3, 4)
        ).reshape(nsteps, PB, CHUNK)
        msk = np.zeros((PB, 1), dtype=np.float32)
        msk[0:NX, 0] = 0.0 if c == 0 else 1.0           # bottom strip valid?
        msk[NX:PB, 0] = 0.0 if c == NCORES - 1 else 1.0  # top strip valid?
        in_maps.append({"w0": wc, "xin": xc, "msk": msk})
    return in_maps


def kernel(external_input, edge_values, edge_rows, edge_cols, num_steps):
    nsteps = int(num_steps)
    if nsteps not in _CACHE:
        _CACHE[nsteps] = _build_graph(nsteps)
    nc = _CACHE[nsteps]

    in_maps = _shard_inputs(external_input, edge_values, edge_rows, edge_cols, nsteps)
    res = bass_utils.run_bass_kernel_spmd(
        nc,
        in_maps,
        core_ids=list(range(NCORES)),
        trace=bool(int(os.environ.get("BRAIN_TRACE", "0"))),
    )

    out = np.empty((nsteps, NX, NCORES, 2, SUB, NZ), dtype=np.float32)
    for c in range(NCORES):
        out[:, :, c] = (
            res.results[c]["spk"]
            .reshape(nsteps, 2, NX, SUB, NZ)
            .transpose(0, 2, 1, 3, 4)
        )
    kernel.last_results = res
    return out.reshape(nsteps, N)
